# revision 13
# baseline (speedup 1.0000x reference)
"""Performer attention (FAVOR+) Bass/Tile kernel for TRN2, SPMD over 8 cores.

Sharding: core i handles batch b = i // 4 and head-group hg = i % 4
(4 heads of 16).  Each core computes its heads' attention output and a
partial output projection [T, D_MODEL]; the host sums the 4 partials per
batch and adds the output bias.

Math notes:
  - The Performer feature map is exp(xp - diag - max(xp - diag)) + eps
    with xp = (q * dn) @ proj.T.  diag is constant along the feature
    axis, so max(xp - diag) = max(xp) - diag and the exp argument is
    exactly xp - max(xp): diag cancels and is never computed.
  - q itself is never needed, only xp — so the host fuses the Q/K
    projection with the random-feature projection:
    xp = x @ (dn * wq_h.T @ proj.T), one [1024 -> 64] matmul per head.
  - The causal cumsum scan is chunked (chunk C=128):
        O_c = tril(Q'_c K'_c^T) Vaug_c + Q'_c S_{c-1},
        S_c = S_{c-1} + K'_c^T Vaug_c,
    with Vaug = [V, 1] so the denominator rides along as column 64.
  - num/den division is applied per row before the output projection.
"""

import numpy as np

import concourse.bacc as bacc
import concourse.mybir as mybir
import concourse.tile as tile
from concourse.bass_utils import run_bass_kernel_spmd
from concourse.masks import make_identity, make_upper_triangular

F32 = mybir.dt.float32
F32R = mybir.dt.float32r

D_MODEL = 1024
D = 64          # head dim
M = 64          # random features
B = 2
T = 1024
NCORES = 8
HG = 4          # heads per core
NCH = T // 128  # 8 t-chunks
KCH = D_MODEL // 128  # 8 contraction chunks for projections
DN = 1.0 / np.sqrt(np.sqrt(np.float32(D)))

# float32r (tf32-like single-pass matmul, ~4x faster at N>=256) for the
# big projection matmuls.  Off by default: costs ~1e-4 relative error.
F32R_PROJ = False


def build_nc(f32r_proj=F32R_PROJ):
    nc = bacc.Bacc("TRN2", target_bir_lowering=False, debug=False)
    WD = F32R if f32r_proj else F32

    xT_d = nc.dram_tensor("xT", [D_MODEL, T], WD, kind="ExternalInput").ap()
    wqe_d = nc.dram_tensor("wqe", [D_MODEL, HG * M], WD, kind="ExternalInput").ap()
    wke_d = nc.dram_tensor("wke", [D_MODEL, HG * M], WD, kind="ExternalInput").ap()
    wvT_d = nc.dram_tensor("wvT", [D_MODEL, HG * D], WD, kind="ExternalInput").ap()
    bqe_d = nc.dram_tensor("bqe", [1, HG * M], F32, kind="ExternalInput").ap()
    bke_d = nc.dram_tensor("bke", [1, HG * M], F32, kind="ExternalInput").ap()
    bv_d = nc.dram_tensor("bv", [1, HG * D], F32, kind="ExternalInput").ap()
    woT_d = nc.dram_tensor("woT", [HG * D, D_MODEL], WD, kind="ExternalInput").ap()
    out_d = nc.dram_tensor("out_p", [T, D_MODEL], F32, kind="ExternalOutput").ap()

    with tile.TileContext(nc) as tc:
        with (
            tc.tile_pool(name="singles", bufs=1) as sg,
            tc.tile_pool(name="scratch", bufs=3) as sc,
            tc.tile_pool(name="ps", bufs=8, space="PSUM") as ps,
        ):
            # ---- persistent SBUF tiles ----
            xT_sb = sg.tile([128, KCH * T], WD, tag="xT", name="xT_sb")
            w_sb = {
                "q": sg.tile([128, KCH * HG * M], WD, tag="wq", name="wq_sb"),
                "k": sg.tile([128, KCH * HG * M], WD, tag="wk", name="wk_sb"),
                "v": sg.tile([128, KCH * HG * D], WD, tag="wv", name="wv_sb"),
            }
            # biases broadcast to 128 partitions (added on DVE during the
            # PSUM->SBUF piece copy; ACT bias can't vary along free dim)
            bb_sb = {
                "q": sg.tile([128, HG * M], F32, tag="bbq", name="bbq_sb"),
                "k": sg.tile([128, HG * M], F32, tag="bbk", name="bbk_sb"),
                "v": sg.tile([128, HG * D], F32, tag="bbv", name="bbv_sb"),
            }
            woc_sb = [sg.tile([128, D_MODEL], WD, tag=f"woc{p}", name=f"woc_sb{p}")
                      for p in range(2)]
            mask_sb = sg.tile([128, 128], F32, tag="mask")
            ident_sb = sg.tile([128, 128], F32, tag="ident")

            kp_sb = [sg.tile([128, HG * D], F32, tag=f"kp{c}", name=f"kp_sb{c}")
                     for c in range(NCH)]
            vaug_sb = [sg.tile([128, HG * (D + 1)], F32, tag=f"va{c}",
                               name=f"va_sb{c}") for c in range(NCH)]
            qpT_sb = [sg.tile([128, T], F32, tag=f"qpT{p}", name=f"qpT_sb{p}")
                      for p in range(2)]
            kpT_sb = [sg.tile([128, T], F32, tag=f"kpT{p}", name=f"kpT_sb{p}")
                      for p in range(2)]
            yt_sb = [sg.tile([128, T], WD, tag=f"yt{p}", name=f"yt_sb{p}")
                     for p in range(2)]

            # ---- constants / DMAs in ----
            make_upper_triangular(nc, mask_sb, val=1.0, diag=True)
            make_identity(nc, ident_sb)
            for key, bd, n in (("q", bqe_d, HG * M), ("k", bke_d, HG * M),
                               ("v", bv_d, HG * D)):
                nc.sync.dma_start(out=bb_sb[key], in_=bd.broadcast_to([128, n]))
            # k-major interleave so the k=0 operands of every projection land
            # first and the PE can start accumulating within ~2us
            for k in range(KCH):
                nc.sync.dma_start(out=xT_sb[:, k * T:(k + 1) * T],
                                  in_=xT_d[k * 128:(k + 1) * 128, :])
                for key, wd, n in (("q", wqe_d, HG * M), ("k", wke_d, HG * M),
                                   ("v", wvT_d, HG * D)):
                    nc.sync.dma_start(out=w_sb[key][:, k * n:(k + 1) * n],
                                      in_=wd[k * 128:(k + 1) * 128, :])
            for p in range(2):
                nc.sync.dma_start(out=woc_sb[p], in_=woT_d[p * 128:(p + 1) * 128, :])
            for c in range(NCH):
                for h in range(HG):
                    nc.vector.memset(vaug_sb[c][:, h * 65 + 64:h * 65 + 65], 1.0)

            # ---- phase A+B: fused projections + feature maps, per t-chunk ----
            def proj_piece(key, tc_i, n):
                """PSUM piece [128, n] = (x @ W) for t-chunk tc_i."""
                pps = ps.tile([128, n], F32, tag="ps", name=f"pp_{key}{tc_i}")
                for k in range(KCH):
                    nc.tensor.matmul(
                        pps,
                        xT_sb[:, k * T + tc_i * 128:k * T + (tc_i + 1) * 128],
                        w_sb[key][:, k * n:(k + 1) * n],
                        start=(k == 0), stop=(k == KCH - 1))
                return pps

            def featmap(c, key, dstT):
                """Feature map for chunk c from the xp projection piece."""
                xps = proj_piece(key, c, HG * M)
                xsb = sc.tile([128, HG * M], F32, tag="xsb", name="xsb")
                nc.vector.tensor_add(xsb, xps, bb_sb[key])
                nats = []
                for pair in range(2):
                    po = pair * 128
                    nmx = sc.tile([128, 2], F32, tag="nmx", name="nmx")
                    nc.vector.tensor_reduce(
                        nmx,
                        xsb.rearrange("p (h m) -> p h m", h=HG)
                        [:, 2 * pair:2 * pair + 2, :],
                        axis=mybir.AxisListType.X,
                        op=mybir.AluOpType.max, negate=True)
                    if key == "k":
                        nat = kp_sb[c][:, po:po + 128]
                    else:
                        nat = sc.tile([128, 128], F32, tag="qnat", name="qnat")
                    for i in range(2):
                        nc.scalar.activation(
                            nat[:, i * 64:(i + 1) * 64],
                            xsb[:, po + i * 64:po + (i + 1) * 64],
                            mybir.ActivationFunctionType.Exp,
                            bias=nmx[:, i:i + 1])
                    if key == "k":
                        # eps must be in the natural copy too (state mm)
                        nc.vector.tensor_scalar_add(nat, nat, 1e-6)
                    nats.append(nat)
                return nats

            def transpose_featmap(c, key, nats, dstT):
                for pair in range(2):
                    tp = ps.tile([128, 128], F32, tag="ps", name="tp")
                    nc.tensor.transpose(tp, nats[pair], ident_sb)
                    if key == "k":
                        nc.scalar.copy(dstT[pair][:, c * 128:(c + 1) * 128], tp)
                    else:
                        nc.vector.tensor_scalar_add(
                            dstT[pair][:, c * 128:(c + 1) * 128], tp, 1e-6)

            def scan_chunk(c, s_prev):
                """Causal chunked scan for chunk c; returns new state tiles."""
                ats, sts, s_out = [], [], [None] * HG
                for h in range(HG):
                    pair, po = h // 2, (h % 2) * 64
                    kpT_c = kpT_sb[pair][po:po + 64, c * 128:(c + 1) * 128]
                    qpT_c = qpT_sb[pair][po:po + 64, c * 128:(c + 1) * 128]
                    at = ps.tile([128, 128], F32, tag="ps", name="at")
                    nc.tensor.matmul(at, kpT_c, qpT_c)
                    ats.append(at)
                for h in range(HG):
                    po = (h % 2) * 64
                    vau = vaug_sb[c][:, h * 65:(h + 1) * 65]
                    # S state lives at the head's partition base so it can be
                    # the rhs of the inter matmul (base must match lhsT).
                    st = ps.tile([128, D + 1], F32, tag="ps", name="st")
                    nc.tensor.matmul(st[po:po + 64, :],
                                     kp_sb[c][:, h * D:(h + 1) * D], vau)
                    sts.append(st)
                atms = []
                for h in range(HG):
                    atm = sc.tile([128, 128], F32, tag="atm", name="atm")
                    nc.vector.tensor_mul(atm, ats[h], mask_sb)
                    atms.append(atm)
                ypair = [sc.tile([128, 128], WD, tag=f"y{p}", name=f"y{p}")
                         for p in range(2)]
                for h in range(HG):
                    pair, po = h // 2, (h % 2) * 64
                    qpT_c = qpT_sb[pair][po:po + 64, c * 128:(c + 1) * 128]
                    vau = vaug_sb[c][:, h * 65:(h + 1) * 65]
                    o = ps.tile([128, D + 1], F32, tag="ps", name="o")
                    if c == 0:
                        nc.tensor.matmul(o, atms[h], vau)
                    else:
                        nc.tensor.matmul(o, atms[h], vau, start=True, stop=False)
                        nc.tensor.matmul(o, qpT_c, s_prev[h][po:po + 64, :],
                                         start=False, stop=True)
                    s_new = sc.tile([128, D + 1], F32, tag=f"s{h}", name=f"s{h}")
                    if c == 0:
                        nc.scalar.copy(s_new[po:po + 64, :], sts[h][po:po + 64, :])
                    else:
                        nc.vector.tensor_add(s_new[po:po + 64, :],
                                             s_prev[h][po:po + 64, :],
                                             sts[h][po:po + 64, :])
                    s_out[h] = s_new

                    r = sc.tile([128, 1], F32, tag="r", name="r")
                    nc.vector.tensor_scalar_add(r, o[:, D:D + 1], 1e-6)
                    nc.vector.reciprocal(r, r)
                    nc.vector.tensor_scalar_mul(
                        ypair[pair][:, po:po + 64], o[:, 0:D], r)
                for pair in range(2):
                    ytp = ps.tile([128, 128], WD, tag="ps", name="ytp")
                    nc.tensor.transpose(ytp, ypair[pair], ident_sb)
                    nc.scalar.copy(yt_sb[pair][:, c * 128:(c + 1) * 128], ytp)
                return s_out

            def out_proj_chunk(tc_i):
                for hf in range(2):
                    op = ps.tile([128, 512], F32, tag="ps", name="op")
                    nc.tensor.matmul(op, yt_sb[0][:, tc_i * 128:(tc_i + 1) * 128],
                                     woc_sb[0][:, hf * 512:(hf + 1) * 512],
                                     start=True, stop=False)
                    nc.tensor.matmul(op, yt_sb[1][:, tc_i * 128:(tc_i + 1) * 128],
                                     woc_sb[1][:, hf * 512:(hf + 1) * 512],
                                     start=False, stop=True)
                    ost = sc.tile([128, 512], F32, tag="ost", name="ost")
                    nc.any.tensor_copy(ost, op)
                    nc.sync.dma_start(
                        out=out_d[tc_i * 128:(tc_i + 1) * 128,
                                  hf * 512:(hf + 1) * 512],
                        in_=ost)

            # One pipelined pass per t-chunk: projections of chunk c fill the
            # PE while chunk c-1's scan/output chains drain, keeping the PE
            # dense (HAM stays at full clock).
            s_prev = [None] * HG
            for c in range(NCH):
                qnats = featmap(c, "q", qpT_sb)
                knats = featmap(c, "k", kpT_sb)
                vps = proj_piece("v", c, HG * D)
                va = vaug_sb[c].rearrange("p (h e) -> p h e", h=HG)
                nc.vector.tensor_add(
                    va[:, :, 0:D],
                    vps.rearrange("p (h e) -> p h e", h=HG),
                    bb_sb["v"].rearrange("p (h e) -> p h e", h=HG))
                transpose_featmap(c, "q", qnats, qpT_sb)
                transpose_featmap(c, "k", knats, kpT_sb)
                if c > 0:
                    out_proj_chunk(c - 1)
                s_prev = scan_chunk(c, s_prev)
            out_proj_chunk(NCH - 1)
    nc.compile()
    return nc


_NC = None
LAST_RESULTS = None


def _f32(a):
    return np.asarray(a, np.float32)


def make_in_maps(x, wq, bq, wk, bk, wv, bv, wo, bo, proj):
    x = _f32(x)
    projT = _f32(proj).astype(np.float64).T  # [D, M]
    xT = [np.ascontiguousarray(x[b].T) for b in range(B)]

    group_maps = []
    for hg in range(4):
        rows = slice(hg * HG * D, (hg + 1) * HG * D)

        def eff(w, bias):
            # per-head fused projection: dn * w_h.T @ proj.T
            wr = _f32(w).astype(np.float64)[rows]          # [256, 1024]
            br = _f32(bias).astype(np.float64)[rows]       # [256]
            wcols, bcols = [], []
            for h in range(HG):
                wh = wr[h * D:(h + 1) * D]                 # [64, 1024]
                bh = br[h * D:(h + 1) * D]
                wcols.append(DN * (wh.T @ projT))          # [1024, 64]
                bcols.append(DN * (bh @ projT))            # [64]
            return (np.ascontiguousarray(
                        np.concatenate(wcols, 1).astype(np.float32)),
                    np.ascontiguousarray(
                        np.concatenate(bcols)[None].astype(np.float32)))

        wqe, bqe = eff(wq, bq)
        wke, bke = eff(wk, bk)
        group_maps.append({
            "wqe": wqe, "bqe": bqe,
            "wke": wke, "bke": bke,
            "wvT": np.ascontiguousarray(_f32(wv)[rows].T),
            "bv": np.ascontiguousarray(_f32(bv)[rows][None]),
            "woT": np.ascontiguousarray(_f32(wo)[:, rows].T),
        })

    return [dict(group_maps[core % 4], xT=xT[core // 4])
            for core in range(NCORES)]


def kernel(x, wq, bq, wk, bk, wv, bv, wo, bo, proj, **run_kwargs):
    global _NC, LAST_RESULTS
    if _NC is None:
        _NC = build_nc()
    in_maps = make_in_maps(x, wq, bq, wk, bk, wv, bv, wo, bo, proj)
    res = run_bass_kernel_spmd(_NC, in_maps, list(range(NCORES)), **run_kwargs)
    LAST_RESULTS = res
    bo = _f32(bo)
    parts = [res.results[i]["out_p"] for i in range(NCORES)]
    out = np.empty((B, T, D_MODEL), np.float32)
    for b in range(B):
        acc = parts[4 * b].copy()
        for i in range(1, 4):
            acc += parts[4 * b + i]
        out[b] = acc + bo[None, :]
    return out


# revision 16
# speedup vs baseline: 1.0261x; 1.0261x over previous
"""Performer attention (FAVOR+) Bass/Tile kernel for TRN2, SPMD over 8 cores.

Sharding: core i handles batch b = i // 4 and head-group hg = i % 4
(4 heads of 16).  Each core computes its heads' attention output and a
partial output projection [T, D_MODEL]; the host sums the 4 partials per
batch and adds the output bias.

Math notes:
  - The Performer feature map is exp(xp - diag - max(xp - diag)) + eps
    with xp = (q * dn) @ proj.T.  diag is constant along the feature
    axis, so max(xp - diag) = max(xp) - diag and the exp argument is
    exactly xp - max(xp): diag cancels and is never computed.
  - q itself is never needed, only xp — so the host fuses the Q/K
    projection with the random-feature projection:
    xp = x @ (dn * wq_h.T @ proj.T), one [1024 -> 64] matmul per head.
  - The causal cumsum scan is chunked (chunk C=128):
        O_c = tril(Q'_c K'_c^T) Vaug_c + Q'_c S_{c-1},
        S_c = S_{c-1} + K'_c^T Vaug_c,
    with Vaug = [V, 1] so the denominator rides along as column 64.
  - num/den division is applied per row before the output projection.
"""

import numpy as np

import concourse.bacc as bacc
import concourse.mybir as mybir
import concourse.tile as tile
from concourse.bass_utils import run_bass_kernel_spmd
from concourse.masks import make_identity, make_upper_triangular

F32 = mybir.dt.float32
F32R = mybir.dt.float32r

D_MODEL = 1024
D = 64          # head dim
M = 64          # random features
B = 2
T = 1024
NCORES = 8
HG = 4          # heads per core
NCH = T // 128  # 8 t-chunks
KCH = D_MODEL // 128  # 8 contraction chunks for projections
DN = 1.0 / np.sqrt(np.sqrt(np.float32(D)))

# float32r (tf32-like single-pass matmul, ~4x faster at N>=256) for the
# big projection matmuls.  Off by default: costs ~1e-4 relative error.
F32R_PROJ = False


def build_nc(f32r_proj=F32R_PROJ):
    nc = bacc.Bacc("TRN2", target_bir_lowering=False, debug=False)
    WD = F32R if f32r_proj else F32

    xT_d = nc.dram_tensor("xT", [D_MODEL, T], WD, kind="ExternalInput").ap()
    wqe_d = nc.dram_tensor("wqe", [D_MODEL, HG * M], WD, kind="ExternalInput").ap()
    wke_d = nc.dram_tensor("wke", [D_MODEL, HG * M], WD, kind="ExternalInput").ap()
    wvT_d = nc.dram_tensor("wvT", [D_MODEL, HG * D], WD, kind="ExternalInput").ap()
    bqe_d = nc.dram_tensor("bqe", [1, HG * M], F32, kind="ExternalInput").ap()
    bke_d = nc.dram_tensor("bke", [1, HG * M], F32, kind="ExternalInput").ap()
    bv_d = nc.dram_tensor("bv", [1, HG * D], F32, kind="ExternalInput").ap()
    woT_d = nc.dram_tensor("woT", [HG * D, D_MODEL], WD, kind="ExternalInput").ap()
    out_d = nc.dram_tensor("out_p", [T, D_MODEL], F32, kind="ExternalOutput").ap()

    with tile.TileContext(nc) as tc:
        with (
            tc.tile_pool(name="singles", bufs=1) as sg,
            tc.tile_pool(name="scratch", bufs=3) as sc,
            tc.tile_pool(name="ps", bufs=8, space="PSUM") as ps,
        ):
            # ---- persistent SBUF tiles ----
            xT_sb = sg.tile([128, KCH * T], WD, tag="xT", name="xT_sb")
            w_sb = {
                "q": sg.tile([128, KCH * HG * M], WD, tag="wq", name="wq_sb"),
                "k": sg.tile([128, KCH * HG * M], WD, tag="wk", name="wk_sb"),
                "v": sg.tile([128, KCH * HG * D], WD, tag="wv", name="wv_sb"),
            }
            # biases broadcast to 128 partitions (added on DVE during the
            # PSUM->SBUF piece copy; ACT bias can't vary along free dim)
            bb_sb = {
                "q": sg.tile([128, HG * M], F32, tag="bbq", name="bbq_sb"),
                "k": sg.tile([128, HG * M], F32, tag="bbk", name="bbk_sb"),
                "v": sg.tile([128, HG * D], F32, tag="bbv", name="bbv_sb"),
            }
            woc_sb = [sg.tile([128, D_MODEL], WD, tag=f"woc{p}", name=f"woc_sb{p}")
                      for p in range(2)]
            mask_sb = sg.tile([128, 128], F32, tag="mask")
            ident_sb = sg.tile([128, 128], F32, tag="ident")

            kp_sb = [sg.tile([128, HG * D], F32, tag=f"kp{c}", name=f"kp_sb{c}")
                     for c in range(NCH)]
            vaug_sb = [sg.tile([128, HG * (D + 1)], F32, tag=f"va{c}",
                               name=f"va_sb{c}") for c in range(NCH)]
            qpT_sb = [sg.tile([128, T], F32, tag=f"qpT{p}", name=f"qpT_sb{p}")
                      for p in range(2)]
            kpT_sb = [sg.tile([128, T], F32, tag=f"kpT{p}", name=f"kpT_sb{p}")
                      for p in range(2)]
            yt_sb = [sg.tile([128, T], WD, tag=f"yt{p}", name=f"yt_sb{p}")
                     for p in range(2)]

            # ---- constants / DMAs in ----
            make_upper_triangular(nc, mask_sb, val=1.0, diag=True)
            make_identity(nc, ident_sb)
            # k-major interleave so the k=0 operands of every projection land
            # first and the PE can start accumulating within a few us; weights
            # issue on gpsimd so the two DMA issue streams run in parallel.
            for k in range(KCH):
                if k == 0:  # split across two queues to land sooner
                    for hf in range(2):
                        nc.sync.dma_start(
                            out=xT_sb[:, hf * 512:(hf + 1) * 512],
                            in_=xT_d[0:128, hf * 512:(hf + 1) * 512])
                else:
                    nc.sync.dma_start(out=xT_sb[:, k * T:(k + 1) * T],
                                      in_=xT_d[k * 128:(k + 1) * 128, :])
                for key, wd, n in (("q", wqe_d, HG * M), ("k", wke_d, HG * M),
                                   ("v", wvT_d, HG * D)):
                    nc.gpsimd.dma_start(out=w_sb[key][:, k * n:(k + 1) * n],
                                        in_=wd[k * 128:(k + 1) * 128, :])
                if k == 0:
                    for key, bd, n in (("q", bqe_d, HG * M), ("k", bke_d, HG * M),
                                       ("v", bv_d, HG * D)):
                        nc.gpsimd.dma_start(out=bb_sb[key],
                                            in_=bd.broadcast_to([128, n]))
            for p in range(2):
                nc.sync.dma_start(out=woc_sb[p], in_=woT_d[p * 128:(p + 1) * 128, :])
            for c in range(NCH):
                for h in range(HG):
                    nc.vector.memset(vaug_sb[c][:, h * 65 + 64:h * 65 + 65], 1.0)

            # ---- phase A+B: fused projections + feature maps, per t-chunk ----
            def proj_piece(key, tc_i, n):
                """PSUM piece [128, n] = (x @ W) for t-chunk tc_i."""
                pps = ps.tile([128, n], F32, tag="ps", name=f"pp_{key}{tc_i}")
                for k in range(KCH):
                    nc.tensor.matmul(
                        pps,
                        xT_sb[:, k * T + tc_i * 128:k * T + (tc_i + 1) * 128],
                        w_sb[key][:, k * n:(k + 1) * n],
                        start=(k == 0), stop=(k == KCH - 1))
                return pps

            def featmap(c, key, dstT):
                """Feature map for chunk c from the xp projection piece."""
                xps = proj_piece(key, c, HG * M)
                xsb = sc.tile([128, HG * M], F32, tag="xsb", name="xsb")
                nc.vector.tensor_add(xsb, xps, bb_sb[key])
                nats = []
                for pair in range(2):
                    po = pair * 128
                    nmx = sc.tile([128, 2], F32, tag="nmx", name="nmx")
                    nc.vector.tensor_reduce(
                        nmx,
                        xsb.rearrange("p (h m) -> p h m", h=HG)
                        [:, 2 * pair:2 * pair + 2, :],
                        axis=mybir.AxisListType.X,
                        op=mybir.AluOpType.max, negate=True)
                    if key == "k":
                        nat = kp_sb[c][:, po:po + 128]
                    else:
                        nat = sc.tile([128, 128], F32, tag="qnat", name="qnat")
                    for i in range(2):
                        nc.scalar.activation(
                            nat[:, i * 64:(i + 1) * 64],
                            xsb[:, po + i * 64:po + (i + 1) * 64],
                            mybir.ActivationFunctionType.Exp,
                            bias=nmx[:, i:i + 1])
                    if key == "k":
                        # eps must be in the natural copy too (state mm)
                        nc.vector.tensor_scalar_add(nat, nat, 1e-6)
                    nats.append(nat)
                return nats

            def transpose_featmap(c, key, nats, dstT):
                for pair in range(2):
                    tp = ps.tile([128, 128], F32, tag="ps", name="tp")
                    nc.tensor.transpose(tp, nats[pair], ident_sb)
                    if key == "k":
                        nc.scalar.copy(dstT[pair][:, c * 128:(c + 1) * 128], tp)
                    else:
                        nc.vector.tensor_scalar_add(
                            dstT[pair][:, c * 128:(c + 1) * 128], tp, 1e-6)

            def scan_chunk(c, s_prev):
                """Causal chunked scan for chunk c; returns new state tiles."""
                ats, sts, s_out = [], [], [None] * HG
                for h in range(HG):
                    pair, po = h // 2, (h % 2) * 64
                    kpT_c = kpT_sb[pair][po:po + 64, c * 128:(c + 1) * 128]
                    qpT_c = qpT_sb[pair][po:po + 64, c * 128:(c + 1) * 128]
                    at = ps.tile([128, 128], F32, tag="ps", name="at")
                    nc.tensor.matmul(at, kpT_c, qpT_c)
                    ats.append(at)
                for h in range(HG):
                    po = (h % 2) * 64
                    vau = vaug_sb[c][:, h * 65:(h + 1) * 65]
                    # S state lives at the head's partition base so it can be
                    # the rhs of the inter matmul (base must match lhsT).
                    st = ps.tile([128, D + 1], F32, tag="ps", name="st")
                    nc.tensor.matmul(st[po:po + 64, :],
                                     kp_sb[c][:, h * D:(h + 1) * D], vau)
                    sts.append(st)
                atms = []
                for h in range(HG):
                    atm = sc.tile([128, 128], F32, tag="atm", name="atm")
                    nc.vector.tensor_mul(atm, ats[h], mask_sb)
                    atms.append(atm)
                ypair = [sc.tile([128, 128], WD, tag=f"y{p}", name=f"y{p}")
                         for p in range(2)]
                for h in range(HG):
                    pair, po = h // 2, (h % 2) * 64
                    qpT_c = qpT_sb[pair][po:po + 64, c * 128:(c + 1) * 128]
                    vau = vaug_sb[c][:, h * 65:(h + 1) * 65]
                    o = ps.tile([128, D + 1], F32, tag="ps", name="o")
                    if c == 0:
                        nc.tensor.matmul(o, atms[h], vau)
                    else:
                        nc.tensor.matmul(o, atms[h], vau, start=True, stop=False)
                        nc.tensor.matmul(o, qpT_c, s_prev[h][po:po + 64, :],
                                         start=False, stop=True)
                    s_new = sc.tile([128, D + 1], F32, tag=f"s{h}", name=f"s{h}")
                    if c == 0:
                        nc.scalar.copy(s_new[po:po + 64, :], sts[h][po:po + 64, :])
                    else:
                        nc.vector.tensor_add(s_new[po:po + 64, :],
                                             s_prev[h][po:po + 64, :],
                                             sts[h][po:po + 64, :])
                    s_out[h] = s_new

                    r = sc.tile([128, 1], F32, tag="r", name="r")
                    nc.vector.tensor_scalar_add(r, o[:, D:D + 1], 1e-6)
                    nc.vector.reciprocal(r, r)
                    nc.vector.tensor_scalar_mul(
                        ypair[pair][:, po:po + 64], o[:, 0:D], r)
                return s_out, ypair

            def transpose_y(c, ypair):
                for pair in range(2):
                    ytp = ps.tile([128, 128], WD, tag="ps", name="ytp")
                    nc.tensor.transpose(ytp, ypair[pair], ident_sb)
                    nc.scalar.copy(yt_sb[pair][:, c * 128:(c + 1) * 128], ytp)

            def out_proj_chunk(tc_i):
                for hf in range(2):
                    op = ps.tile([128, 512], F32, tag="ps", name="op")
                    nc.tensor.matmul(op, yt_sb[0][:, tc_i * 128:(tc_i + 1) * 128],
                                     woc_sb[0][:, hf * 512:(hf + 1) * 512],
                                     start=True, stop=False)
                    nc.tensor.matmul(op, yt_sb[1][:, tc_i * 128:(tc_i + 1) * 128],
                                     woc_sb[1][:, hf * 512:(hf + 1) * 512],
                                     start=False, stop=True)
                    ost = sc.tile([128, 512], F32, tag="ost", name="ost")
                    nc.any.tensor_copy(ost, op)
                    nc.sync.dma_start(
                        out=out_d[tc_i * 128:(tc_i + 1) * 128,
                                  hf * 512:(hf + 1) * 512],
                        in_=ost)

            # One pipelined pass per t-chunk: projections of chunk c fill the
            # PE while chunk c-1's scan/output chains drain, keeping the PE
            # dense (HAM stays at full clock).  Chunk c-1's y-transposes and
            # output projection are deferred into iteration c so their DVE
            # producers get a full chunk of slack before the PE needs them.
            s_prev = [None] * HG
            yprev = None
            for c in range(NCH):
                qnats = featmap(c, "q", qpT_sb)
                knats = featmap(c, "k", kpT_sb)
                vps = proj_piece("v", c, HG * D)
                va = vaug_sb[c].rearrange("p (h e) -> p h e", h=HG)
                nc.vector.tensor_add(
                    va[:, :, 0:D],
                    vps.rearrange("p (h e) -> p h e", h=HG),
                    bb_sb["v"].rearrange("p (h e) -> p h e", h=HG))
                transpose_featmap(c, "q", qnats, qpT_sb)
                transpose_featmap(c, "k", knats, kpT_sb)
                if c > 0:
                    transpose_y(c - 1, yprev)
                    out_proj_chunk(c - 1)
                s_prev, yprev = scan_chunk(c, s_prev)
            transpose_y(NCH - 1, yprev)
            out_proj_chunk(NCH - 1)
    nc.compile()
    return nc


_NC = None
LAST_RESULTS = None


def _f32(a):
    return np.asarray(a, np.float32)


def make_in_maps(x, wq, bq, wk, bk, wv, bv, wo, bo, proj):
    x = _f32(x)
    projT = _f32(proj).astype(np.float64).T  # [D, M]
    xT = [np.ascontiguousarray(x[b].T) for b in range(B)]

    group_maps = []
    for hg in range(4):
        rows = slice(hg * HG * D, (hg + 1) * HG * D)

        def eff(w, bias):
            # per-head fused projection: dn * w_h.T @ proj.T
            wr = _f32(w).astype(np.float64)[rows]          # [256, 1024]
            br = _f32(bias).astype(np.float64)[rows]       # [256]
            wcols, bcols = [], []
            for h in range(HG):
                wh = wr[h * D:(h + 1) * D]                 # [64, 1024]
                bh = br[h * D:(h + 1) * D]
                wcols.append(DN * (wh.T @ projT))          # [1024, 64]
                bcols.append(DN * (bh @ projT))            # [64]
            return (np.ascontiguousarray(
                        np.concatenate(wcols, 1).astype(np.float32)),
                    np.ascontiguousarray(
                        np.concatenate(bcols)[None].astype(np.float32)))

        wqe, bqe = eff(wq, bq)
        wke, bke = eff(wk, bk)
        group_maps.append({
            "wqe": wqe, "bqe": bqe,
            "wke": wke, "bke": bke,
            "wvT": np.ascontiguousarray(_f32(wv)[rows].T),
            "bv": np.ascontiguousarray(_f32(bv)[rows][None]),
            "woT": np.ascontiguousarray(_f32(wo)[:, rows].T),
        })

    return [dict(group_maps[core % 4], xT=xT[core // 4])
            for core in range(NCORES)]


def kernel(x, wq, bq, wk, bk, wv, bv, wo, bo, proj, **run_kwargs):
    global _NC, LAST_RESULTS
    if _NC is None:
        _NC = build_nc()
    in_maps = make_in_maps(x, wq, bq, wk, bk, wv, bv, wo, bo, proj)
    res = run_bass_kernel_spmd(_NC, in_maps, list(range(NCORES)), **run_kwargs)
    LAST_RESULTS = res
    bo = _f32(bo)
    parts = [res.results[i]["out_p"] for i in range(NCORES)]
    out = np.empty((B, T, D_MODEL), np.float32)
    for b in range(B):
        acc = parts[4 * b].copy()
        for i in range(1, 4):
            acc += parts[4 * b + i]
        out[b] = acc + bo[None, :]
    return out


# revision 18
# speedup vs baseline: 1.5456x; 1.5063x over previous
"""Performer attention (FAVOR+) Bass/Tile kernel for TRN2, SPMD over 8 cores.

Sharding: core i handles batch b = i // 4 and head-group hg = i % 4
(4 heads of 16).  Each core computes its heads' attention output and a
partial output projection [T, D_MODEL]; the host sums the 4 partials per
batch and adds the output bias.

Math notes:
  - The Performer feature map is exp(xp - diag - max(xp - diag)) + eps
    with xp = (q * dn) @ proj.T.  diag is constant along the feature
    axis, so max(xp - diag) = max(xp) - diag and the exp argument is
    exactly xp - max(xp): diag cancels and is never computed.
  - q itself is never needed, only xp — so the host fuses the Q/K
    projection with the random-feature projection:
    xp = x @ (dn * wq_h.T @ proj.T), one [1024 -> 64] matmul per head.
  - The causal cumsum scan is chunked (chunk C=128):
        O_c = tril(Q'_c K'_c^T) Vaug_c + Q'_c S_{c-1},
        S_c = S_{c-1} + K'_c^T Vaug_c,
    with Vaug = [V, 1] so the denominator rides along as column 64.
  - num/den division is applied per row before the output projection.
"""

import numpy as np

import concourse.bacc as bacc
import concourse.mybir as mybir
import concourse.tile as tile
from concourse.bass_utils import run_bass_kernel_spmd
from concourse.masks import make_identity, make_upper_triangular

F32 = mybir.dt.float32
F32R = mybir.dt.float32r

D_MODEL = 1024
D = 64          # head dim
M = 64          # random features
B = 2
T = 1024
NCORES = 8
HG = 4          # heads per core
NCH = T // 128  # 8 t-chunks
KCH = D_MODEL // 128  # 8 contraction chunks for projections
DN = 1.0 / np.sqrt(np.sqrt(np.float32(D)))

# float32r (tf32-like single-pass matmul, ~4x faster at N>=256) for the
# big projection matmuls.  Off by default: costs ~1e-4 relative error.
F32R_PROJ = False


def build_nc(f32r_proj=F32R_PROJ):
    nc = bacc.Bacc("TRN2", target_bir_lowering=False, debug=False)
    WD = F32R if f32r_proj else F32

    xT_d = nc.dram_tensor("xT", [D_MODEL, T], WD, kind="ExternalInput").ap()
    wqe_d = nc.dram_tensor("wqe", [D_MODEL, HG * M], WD, kind="ExternalInput").ap()
    wke_d = nc.dram_tensor("wke", [D_MODEL, HG * M], WD, kind="ExternalInput").ap()
    wvT_d = nc.dram_tensor("wvT", [D_MODEL, HG * D], WD, kind="ExternalInput").ap()
    bqe_d = nc.dram_tensor("bqe", [1, HG * M], F32, kind="ExternalInput").ap()
    bke_d = nc.dram_tensor("bke", [1, HG * M], F32, kind="ExternalInput").ap()
    bv_d = nc.dram_tensor("bv", [1, HG * D], F32, kind="ExternalInput").ap()
    woT_d = nc.dram_tensor("woT", [HG * D, D_MODEL], WD, kind="ExternalInput").ap()
    out_d = nc.dram_tensor("out_p", [T, D_MODEL], F32, kind="ExternalOutput").ap()

    with tile.TileContext(nc) as tc:
        with (
            tc.tile_pool(name="singles", bufs=1) as sg,
            tc.tile_pool(name="scratch", bufs=3) as sc,
            tc.tile_pool(name="ps", bufs=8, space="PSUM") as ps,
        ):
            # ---- persistent SBUF tiles ----
            xT_sb = sg.tile([128, KCH * T], WD, tag="xT", name="xT_sb")
            w_sb = {
                "q": sg.tile([128, KCH * HG * M], WD, tag="wq", name="wq_sb"),
                "k": sg.tile([128, KCH * HG * M], WD, tag="wk", name="wk_sb"),
                "v": sg.tile([128, KCH * HG * D], WD, tag="wv", name="wv_sb"),
            }
            # biases broadcast to 128 partitions (added on DVE during the
            # PSUM->SBUF piece copy; ACT bias can't vary along free dim)
            bb_sb = {
                "q": sg.tile([128, HG * M], F32, tag="bbq", name="bbq_sb"),
                "k": sg.tile([128, HG * M], F32, tag="bbk", name="bbk_sb"),
                "v": sg.tile([128, HG * D], F32, tag="bbv", name="bbv_sb"),
            }
            woc_sb = [sg.tile([128, D_MODEL], WD, tag=f"woc{p}", name=f"woc_sb{p}")
                      for p in range(2)]
            mask_sb = sg.tile([128, 128], F32, tag="mask")
            ident_sb = sg.tile([128, 128], F32, tag="ident")

            kp_sb = [sg.tile([128, HG * D], F32, tag=f"kp{c}", name=f"kp_sb{c}")
                     for c in range(NCH)]
            vaug_sb = [sg.tile([128, HG * (D + 1)], F32, tag=f"va{c}",
                               name=f"va_sb{c}") for c in range(NCH)]
            qpT_sb = [sg.tile([128, T], F32, tag=f"qpT{p}", name=f"qpT_sb{p}")
                      for p in range(2)]
            kpT_sb = [sg.tile([128, T], F32, tag=f"kpT{p}", name=f"kpT_sb{p}")
                      for p in range(2)]
            yt_sb = [sg.tile([128, T], WD, tag=f"yt{p}", name=f"yt_sb{p}")
                     for p in range(2)]

            # ---- constants / DMAs in ----
            make_upper_triangular(nc, mask_sb, val=1.0, diag=True)
            make_identity(nc, ident_sb)
            # k-major interleave so the k=0 operands of every projection land
            # first and the PE can start accumulating within a few us; weights
            # issue on gpsimd so the two DMA issue streams run in parallel.
            for k in range(KCH):
                if k == 0:  # split across two queues to land sooner
                    for hf in range(2):
                        nc.sync.dma_start(
                            out=xT_sb[:, hf * 512:(hf + 1) * 512],
                            in_=xT_d[0:128, hf * 512:(hf + 1) * 512])
                else:
                    nc.sync.dma_start(out=xT_sb[:, k * T:(k + 1) * T],
                                      in_=xT_d[k * 128:(k + 1) * 128, :])
                for key, wd, n in (("q", wqe_d, HG * M), ("k", wke_d, HG * M),
                                   ("v", wvT_d, HG * D)):
                    nc.gpsimd.dma_start(out=w_sb[key][:, k * n:(k + 1) * n],
                                        in_=wd[k * 128:(k + 1) * 128, :])
                if k == 0:
                    for key, bd, n in (("q", bqe_d, HG * M), ("k", bke_d, HG * M),
                                       ("v", bv_d, HG * D)):
                        nc.gpsimd.dma_start(out=bb_sb[key],
                                            in_=bd.broadcast_to([128, n]))
            for p in range(2):
                nc.sync.dma_start(out=woc_sb[p], in_=woT_d[p * 128:(p + 1) * 128, :])
            for c in range(NCH):
                for h in range(HG):
                    nc.vector.memset(vaug_sb[c][:, h * 65 + 64:h * 65 + 65], 1.0)

            # ---- phase A+B: fused projections + feature maps, per t-chunk ----
            def proj_piece(key, tc_i, n):
                """PSUM piece [128, n] = (x @ W) for t-chunk tc_i."""
                pps = ps.tile([128, n], F32, tag="ps", name=f"pp_{key}{tc_i}")
                for k in range(KCH):
                    nc.tensor.matmul(
                        pps,
                        xT_sb[:, k * T + tc_i * 128:k * T + (tc_i + 1) * 128],
                        w_sb[key][:, k * n:(k + 1) * n],
                        start=(k == 0), stop=(k == KCH - 1))
                return pps

            def featmap(c, key, dstT):
                """Feature map for chunk c from the xp projection piece."""
                xps = proj_piece(key, c, HG * M)
                xsb = sc.tile([128, HG * M], F32, tag="xsb", name="xsb")
                nc.vector.tensor_add(xsb, xps, bb_sb[key])
                nats = []
                for pair in range(2):
                    po = pair * 128
                    nmx = sc.tile([128, 2], F32, tag="nmx", name="nmx")
                    nc.vector.tensor_reduce(
                        nmx,
                        xsb.rearrange("p (h m) -> p h m", h=HG)
                        [:, 2 * pair:2 * pair + 2, :],
                        axis=mybir.AxisListType.X,
                        op=mybir.AluOpType.max, negate=True)
                    if key == "k":
                        nat = kp_sb[c][:, po:po + 128]
                    else:
                        nat = sc.tile([128, 128], F32, tag="qnat", name="qnat")
                    for i in range(2):
                        nc.scalar.activation(
                            nat[:, i * 64:(i + 1) * 64],
                            xsb[:, po + i * 64:po + (i + 1) * 64],
                            mybir.ActivationFunctionType.Exp,
                            bias=nmx[:, i:i + 1])
                    if key == "k":
                        # eps must be in the natural copy too (state mm)
                        nc.vector.tensor_scalar_add(nat, nat, 1e-6)
                    nats.append(nat)
                return nats

            def transpose_featmap(c, key, nats, dstT):
                for pair in range(2):
                    tp = ps.tile([128, 128], F32, tag="ps", name="tp")
                    nc.tensor.transpose(tp, nats[pair], ident_sb)
                    if key == "k":
                        nc.scalar.copy(dstT[pair][:, c * 128:(c + 1) * 128], tp)
                    else:
                        nc.vector.tensor_scalar_add(
                            dstT[pair][:, c * 128:(c + 1) * 128], tp, 1e-6)

            def scan_chunk(c, s_prev):
                """Causal chunked scan for chunk c; returns new state tiles."""
                ats, sts, s_out = [], [], [None] * HG
                for h in range(HG):
                    pair, po = h // 2, (h % 2) * 64
                    kpT_c = kpT_sb[pair][po:po + 64, c * 128:(c + 1) * 128]
                    qpT_c = qpT_sb[pair][po:po + 64, c * 128:(c + 1) * 128]
                    at = ps.tile([128, 128], F32, tag="ps", name="at")
                    nc.tensor.matmul(at, kpT_c, qpT_c)
                    ats.append(at)
                for h in range(HG):
                    po = (h % 2) * 64
                    vau = vaug_sb[c][:, h * 65:(h + 1) * 65]
                    # S state lives at the head's partition base so it can be
                    # the rhs of the inter matmul (base must match lhsT).
                    st = ps.tile([128, D + 1], F32, tag="ps", name="st")
                    nc.tensor.matmul(st[po:po + 64, :],
                                     kp_sb[c][:, h * D:(h + 1) * D], vau)
                    sts.append(st)
                atms = []
                for h in range(HG):
                    atm = sc.tile([128, 128], F32, tag="atm", name="atm")
                    nc.vector.tensor_mul(atm, ats[h], mask_sb)
                    atms.append(atm)
                ypair = [sc.tile([128, 128], F32, tag=f"y{p}", name=f"y{p}")
                         for p in range(2)]
                for h in range(HG):
                    pair, po = h // 2, (h % 2) * 64
                    qpT_c = qpT_sb[pair][po:po + 64, c * 128:(c + 1) * 128]
                    vau = vaug_sb[c][:, h * 65:(h + 1) * 65]
                    o = ps.tile([128, D + 1], F32, tag="ps", name="o")
                    if c == 0:
                        nc.tensor.matmul(o, atms[h], vau)
                    else:
                        nc.tensor.matmul(o, atms[h], vau, start=True, stop=False)
                        nc.tensor.matmul(o, qpT_c, s_prev[h][po:po + 64, :],
                                         start=False, stop=True)
                    s_new = sc.tile([128, D + 1], F32, tag=f"s{h}", name=f"s{h}")
                    if c == 0:
                        nc.scalar.copy(s_new[po:po + 64, :], sts[h][po:po + 64, :])
                    else:
                        nc.vector.tensor_add(s_new[po:po + 64, :],
                                             s_prev[h][po:po + 64, :],
                                             sts[h][po:po + 64, :])
                    s_out[h] = s_new

                    r = sc.tile([128, 1], F32, tag="r", name="r")
                    nc.vector.tensor_scalar_add(r, o[:, D:D + 1], 1e-6)
                    nc.vector.reciprocal(r, r)
                    nc.vector.tensor_scalar_mul(
                        ypair[pair][:, po:po + 64], o[:, 0:D], r)
                return s_out, ypair

            def transpose_y(c, ypair):
                for pair in range(2):
                    ytp = ps.tile([128, 128], F32, tag="ps", name="ytp")
                    nc.tensor.transpose(ytp, ypair[pair], ident_sb)
                    nc.scalar.copy(yt_sb[pair][:, c * 128:(c + 1) * 128], ytp)

            def out_proj_chunk(tc_i):
                for hf in range(2):
                    op = ps.tile([128, 512], F32, tag="ps", name="op")
                    nc.tensor.matmul(op, yt_sb[0][:, tc_i * 128:(tc_i + 1) * 128],
                                     woc_sb[0][:, hf * 512:(hf + 1) * 512],
                                     start=True, stop=False)
                    nc.tensor.matmul(op, yt_sb[1][:, tc_i * 128:(tc_i + 1) * 128],
                                     woc_sb[1][:, hf * 512:(hf + 1) * 512],
                                     start=False, stop=True)
                    ost = sc.tile([128, 512], F32, tag="ost", name="ost")
                    nc.any.tensor_copy(ost, op)
                    nc.sync.dma_start(
                        out=out_d[tc_i * 128:(tc_i + 1) * 128,
                                  hf * 512:(hf + 1) * 512],
                        in_=ost)

            # One pipelined pass per t-chunk: projections of chunk c fill the
            # PE while chunk c-1's scan/output chains drain, keeping the PE
            # dense (HAM stays at full clock).  Chunk c-1's y-transposes and
            # output projection are deferred into iteration c so their DVE
            # producers get a full chunk of slack before the PE needs them.
            s_prev = [None] * HG
            yprev = None
            for c in range(NCH):
                qnats = featmap(c, "q", qpT_sb)
                knats = featmap(c, "k", kpT_sb)
                vps = proj_piece("v", c, HG * D)
                va = vaug_sb[c].rearrange("p (h e) -> p h e", h=HG)
                nc.vector.tensor_add(
                    va[:, :, 0:D],
                    vps.rearrange("p (h e) -> p h e", h=HG),
                    bb_sb["v"].rearrange("p (h e) -> p h e", h=HG))
                transpose_featmap(c, "q", qnats, qpT_sb)
                transpose_featmap(c, "k", knats, kpT_sb)
                if c > 0:
                    transpose_y(c - 1, yprev)
                    out_proj_chunk(c - 1)
                s_prev, yprev = scan_chunk(c, s_prev)
            transpose_y(NCH - 1, yprev)
            out_proj_chunk(NCH - 1)
    nc.compile()
    return nc


_NC = None
LAST_RESULTS = None


def _f32(a):
    return np.asarray(a, np.float32)


def make_in_maps(x, wq, bq, wk, bk, wv, bv, wo, bo, proj):
    x = _f32(x)
    projT = _f32(proj).astype(np.float64).T  # [D, M]
    xT = [np.ascontiguousarray(x[b].T) for b in range(B)]

    group_maps = []
    for hg in range(4):
        rows = slice(hg * HG * D, (hg + 1) * HG * D)

        def eff(w, bias):
            # per-head fused projection: dn * w_h.T @ proj.T
            wr = _f32(w).astype(np.float64)[rows]          # [256, 1024]
            br = _f32(bias).astype(np.float64)[rows]       # [256]
            wcols, bcols = [], []
            for h in range(HG):
                wh = wr[h * D:(h + 1) * D]                 # [64, 1024]
                bh = br[h * D:(h + 1) * D]
                wcols.append(DN * (wh.T @ projT))          # [1024, 64]
                bcols.append(DN * (bh @ projT))            # [64]
            return (np.ascontiguousarray(
                        np.concatenate(wcols, 1).astype(np.float32)),
                    np.ascontiguousarray(
                        np.concatenate(bcols)[None].astype(np.float32)))

        wqe, bqe = eff(wq, bq)
        wke, bke = eff(wk, bk)
        group_maps.append({
            "wqe": wqe, "bqe": bqe,
            "wke": wke, "bke": bke,
            "wvT": np.ascontiguousarray(_f32(wv)[rows].T),
            "bv": np.ascontiguousarray(_f32(bv)[rows][None]),
            "woT": np.ascontiguousarray(_f32(wo)[:, rows].T),
        })

    return [dict(group_maps[core % 4], xT=xT[core // 4])
            for core in range(NCORES)]


def kernel(x, wq, bq, wk, bk, wv, bv, wo, bo, proj, **run_kwargs):
    global _NC, LAST_RESULTS
    if _NC is None:
        _NC = build_nc()
    in_maps = make_in_maps(x, wq, bq, wk, bk, wv, bv, wo, bo, proj)
    res = run_bass_kernel_spmd(_NC, in_maps, list(range(NCORES)), **run_kwargs)
    LAST_RESULTS = res
    bo = _f32(bo)
    parts = [res.results[i]["out_p"] for i in range(NCORES)]
    out = np.empty((B, T, D_MODEL), np.float32)
    for b in range(B):
        acc = parts[4 * b].copy()
        for i in range(1, 4):
            acc += parts[4 * b + i]
        out[b] = acc + bo[None, :]
    return out
